# revision 13
# baseline (speedup 1.0000x reference)
"""BiasedMHA Trainium2 kernel: B=8 batches data-parallel across 8 NeuronCores.

Per core (one batch): fused attention with additive bias + boolean mask.
  out = softmax(Q@K^T*scale + bias, mask) @ V @ Wo^T + bo

v4 architecture (engine-specialized, PE kept streaming for p-state):
- host prep: mask folded into bias (-1e30), bias transposed to (q, h, k) so
  each head's stripe is contiguous; weights pre-transposed; ndata
  pre-transposed; everything bf16
- bias is accumulated into the score PSUM via an identity matmul on PE
  (no DVE/Pool bias-add); ACT exp reads PSUM directly
- softmax denominator via DVE tensor_reduce; 1/den folded into e on DVE
- e transposed on the DMA xbar (SP queue); AV batched 4 q-tiles per matmul
- engine roles: PE=matmuls only, ACT=exp + bias-chunk DMA + qt/kt evac,
  DVE=den/recip/scale, Pool=PSUM evacuations, SP=transposes + stores
"""

import sys

import numpy as np

for _p in ("/opt/trn_rl_repo",):
    if _p not in sys.path:
        sys.path.insert(0, _p)

import concourse.bass as bass  # noqa: E402
import concourse.mybir as mybir  # noqa: E402
import concourse.tile as tile  # noqa: E402
from concourse import bacc  # noqa: E402
from concourse.masks import make_identity  # noqa: E402

NN = 1024  # sequence length
F = 256  # feature dim
H = 8  # heads
D = F // H  # head dim = 32
P = 128  # partitions
NT = NN // P  # 8 q/seq tiles
KC = NN // P  # 8 k chunks
FC = F // P  # 2 feature chunks
TB = 4  # q-tiles per AV batch block
NB = NT // TB  # blocks
SCALE = D**-0.5
NEG = -1.0e30

F32 = mybir.dt.float32
BF16 = mybir.dt.bfloat16
AF = mybir.ActivationFunctionType


def build_program():
    """Build the single-core program (one batch). Returns compiled Bacc."""
    nc = bacc.Bacc(
        "TRN2", target_bir_lowering=False, debug=False, num_devices=8
    )

    ndT_dram = nc.dram_tensor("ndT", (F, NN), BF16, kind="ExternalInput").ap()
    bias_dram = nc.dram_tensor(
        "biasT", (NN, H, NN), BF16, kind="ExternalInput"
    ).ap()
    w_dram = {}
    b_dram = {}
    for w in ("q", "k", "v", "o"):
        w_dram[w] = nc.dram_tensor(f"w{w}T", (F, F), BF16, kind="ExternalInput").ap()
        b_dram[w] = nc.dram_tensor(f"b{w}", (F,), F32, kind="ExternalInput").ap()
    out_dram = nc.dram_tensor("out", (NN, F), F32, kind="ExternalOutput").ap()

    with tile.TileContext(nc) as tc:
        _emit(nc, tc, ndT_dram, bias_dram, w_dram, b_dram, out_dram)

    nc.compile()
    return nc


def _emit(nc, tc, ndT_dram, bias_dram, w_dram, b_dram, out_dram):
    from contextlib import ExitStack

    ctx = ExitStack()
    with ctx:
        const = ctx.enter_context(tc.tile_pool(name="const", bufs=1))
        wpool = ctx.enter_context(tc.tile_pool(name="wpool", bufs=1))
        biasp = ctx.enter_context(tc.tile_pool(name="biasp", bufs=3))
        epool = ctx.enter_context(tc.tile_pool(name="epool", bufs=6))
        etp = ctx.enter_context(tc.tile_pool(name="etp", bufs=10))
        small = ctx.enter_context(tc.tile_pool(name="small", bufs=6))
        atp = ctx.enter_context(tc.tile_pool(name="atp", bufs=2))
        ypool = ctx.enter_context(tc.tile_pool(name="ypool", bufs=3))
        psA = ctx.enter_context(tc.tile_pool(name="psA", bufs=3, space="PSUM"))
        psC = ctx.enter_context(tc.tile_pool(name="psC", bufs=2, space="PSUM"))

        # ---- constants ----
        i128 = const.tile([P, P], BF16, tag="i128")
        make_identity(nc, i128)
        ones = const.tile([1, P], BF16, tag="ones")
        nc.vector.memset(ones, 1.0)
        # per-partition projection biases for q/k (f_out = hg*128 + p)
        bcol = {}
        for w in ("q", "k"):
            bcf = const.tile([P, FC], F32, tag=f"b{w}cf")
            nc.sync.dma_start(out=bcf, in_=b_dram[w].rearrange("(c p) -> p c", p=P))
            if w == "q":
                nc.vector.tensor_scalar_mul(bcf, bcf, SCALE)
            bcol[w] = bcf
        # broadcast-row biases for v/o (used via ones-matmul)
        brow = {}
        for w in ("v", "o"):
            bf = const.tile([1, F], F32, tag=f"b{w}f")
            nc.sync.dma_start(out=bf, in_=b_dram[w][None, :])
            bh = const.tile([1, F], BF16, tag=f"b{w}h")
            nc.vector.tensor_copy(bh, bf)
            brow[w] = bh

        # ---- weights + ndata (pre-transposed on host, bf16) ----
        wT = {}
        for w in ("q", "k", "v", "o"):
            wt = wpool.tile([P, FC, F], BF16, tag=f"w{w}T")
            nc.sync.dma_start(
                out=wt, in_=w_dram[w].rearrange("(c p) o -> p c o", p=P)
            )
            wT[w] = wt
        nT = wpool.tile([P, FC, NN], BF16, tag="nT")
        nc.sync.dma_start(out=nT, in_=ndT_dram.rearrange("(c p) n -> p c n", p=P))

        # ---- bias tiles: (q-tile t) -> [P, (h k)] with contiguous per-head k ----
        bias_re = bias_dram.rearrange("(t p) h k -> t p (h k)", p=P)
        bias_tiles = {}
        NCH = 2  # chunks per bias tile (4 heads each)
        CW = NN * H // NCH

        def load_chunk(tt, c, eng=None):
            if tt not in bias_tiles:
                bias_tiles[tt] = biasp.tile(
                    [P, NN * H], BF16, tag="bias", name=f"bias_{tt}"
                )
            (eng or nc.gpsimd).dma_start(
                out=bias_tiles[tt][:, c * CW : (c + 1) * CW],
                in_=bias_re[tt][:, c * CW : (c + 1) * CW],
            )

        # t0 chunks up front (SP queue; ACT during main loop)
        for c in range(NCH):
            load_chunk(0, c, eng=nc.sync)

        # ---- QT/KT projections: head h at partitions 32*(h%4), plane h//4 ----
        qt = wpool.tile([P, H // 4, NN], BF16, tag="qt")
        kt = wpool.tile([P, H // 4, NN], BF16, tag="kt")
        for name, dst, scl in (("q", qt, SCALE), ("k", kt, 1.0)):
            for hg in range(H // 4):
                ps = psA.tile([P, NN], F32, tag="A", name=f"ps_{name}{hg}")
                for j in range(4):
                    h = hg * 4 + j
                    rs = slice(j * D, (j + 1) * D)
                    for qh in range(2):
                        sl = slice(qh * 512, (qh + 1) * 512)
                        for fic in range(FC):
                            nc.tensor.matmul(
                                ps[rs, sl],
                                lhsT=wT[name][:, fic, h * D : (h + 1) * D],
                                rhs=nT[:, fic, sl],
                                start=(fic == 0),
                                stop=(fic == FC - 1),
                                tile_position=(0, j * D),
                            )
                nc.scalar.activation(
                    dst[:, hg, :],
                    ps,
                    AF.Identity,
                    bias=bcol[name][:, hg : hg + 1],
                    scale=scl,
                )

        # ---- V projection: vp[p, kc, f] (seq on partitions) ----
        vp = wpool.tile([P, NT, F], BF16, tag="vp")
        for t in range(NT):
            psv = psC.tile([P, 512], F32, tag="C", name=f"psv_{t}")
            for fic in range(FC):
                nc.tensor.matmul(
                    psv[:, :F],
                    lhsT=nT[:, fic, t * P : (t + 1) * P],
                    rhs=wT["v"][:, fic, :],
                    start=(fic == 0),
                    stop=False,
                )
            nc.tensor.matmul(
                psv[:, :F], lhsT=ones, rhs=brow["v"], start=False, stop=True
            )
            nc.scalar.copy(vp[:, t, :], psv[:, :F])

        # ---- main attention pipeline ----
        # front(g): scores + bias-inject on PE, exp on ACT, den/recip/scale DVE,
        # transpose on SP. back units: AV matmuls batched over TB q-tiles,
        # psc evac on Pool, O-proj + store.
        et_tiles = {}  # (block, head) -> ET tile [P, KC, TB*P]
        at_tiles = {}  # block -> aT tile [P, FC, TB*P]
        psc_tiles = {}  # (block, grp) -> psum tile

        def front(g):
            t, h = divmod(g, H)
            hg, j = h // 4, h % 4
            if t + 1 < NT:
                if h == 0:
                    load_chunk(t + 1, 0)
                elif h == 4:
                    load_chunk(t + 1, 1)
            bias_t = bias_tiles[t]
            psa = psA.tile([P, NN], F32, tag="A", name=f"psa_{g}")
            for kh in range(2):
                sl = slice(kh * 512, (kh + 1) * 512)
                nc.tensor.matmul(
                    psa[:, sl],
                    lhsT=qt[j * D : (j + 1) * D, hg, t * P : (t + 1) * P],
                    rhs=kt[j * D : (j + 1) * D, hg, sl],
                    start=True,
                    stop=False,
                    tile_position=(j * D, 0),
                )
                nc.tensor.matmul(
                    psa[:, sl],
                    lhsT=i128,
                    rhs=bias_t[:, h * NN + kh * 512 : h * NN + (kh + 1) * 512],
                    start=False,
                    stop=True,
                )
            e = epool.tile([P, NN], BF16, tag="e", name=f"e_{g}")
            den = small.tile([P, 1], F32, tag="den", name=f"den_{g}")
            nc.scalar.activation(e, psa, AF.Exp, accum_out=den)
            rec = small.tile([P, 1], F32, tag="rec", name=f"rec_{g}")
            nc.vector.reciprocal(rec, den)
            nc.vector.tensor_scalar_mul(e, e, rec)
            blk, ti = divmod(t, TB)
            key = (blk, h)
            if key not in et_tiles:
                et_tiles[key] = etp.tile(
                    [P, KC, TB * P], BF16, tag="eT", name=f"eT_{blk}_{h}"
                )
            nc.sync.dma_start(
                out=et_tiles[key][:, :, ti * P : (ti + 1) * P], in_=e, transpose=True
            )
            if h == H - 1:
                bias_tiles.pop(t)

        def unit_av(blk, h, quarter):
            """2 AV matmuls (kc pair) for head h over block blk's TB q-tiles."""
            hg, j = h // 4, h % 4
            gi = (blk, hg)
            if gi not in psc_tiles:
                psc_tiles[gi] = psC.tile(
                    [P, TB * P], F32, tag="C", name=f"psc_{blk}_{hg}"
                )
            psc = psc_tiles[gi]
            eT = et_tiles[(blk, h)]
            for kc in range(quarter * 2, quarter * 2 + 2):
                nc.tensor.matmul(
                    psc[j * D : (j + 1) * D, :],
                    lhsT=vp[:, kc, h * D : (h + 1) * D],
                    rhs=eT[:, kc, :],
                    start=(kc == 0),
                    stop=(kc == KC - 1),
                    tile_position=(0, j * D),
                )
            if quarter == 3:
                et_tiles.pop((blk, h))
                if j == 3:
                    if hg == 0:
                        at_tiles[blk] = atp.tile(
                            [P, FC, TB * P], BF16, tag="aT", name=f"aT_{blk}"
                        )
                    nc.vector.tensor_copy(
                        at_tiles[blk][:, hg, :], psc_tiles.pop(gi)
                    )

        def unit_oproj(blk, ti):
            t = blk * TB + ti
            aT = at_tiles[blk]
            psy = psC.tile([P, 512], F32, tag="C", name=f"psy_{t}")
            for fc in range(FC):
                nc.tensor.matmul(
                    psy[:, :F],
                    lhsT=aT[:, fc, ti * P : (ti + 1) * P],
                    rhs=wT["o"][:, fc, :],
                    start=(fc == 0),
                    stop=False,
                )
            nc.tensor.matmul(
                psy[:, :F], lhsT=ones, rhs=brow["o"], start=False, stop=True
            )
            y = ypool.tile([P, F], F32, tag="y", name=f"y_{t}")
            nc.vector.tensor_copy(y, psy[:, :F])
            nc.gpsimd.dma_start(out=out_dram[t * P : (t + 1) * P, :], in_=y)
            if ti == TB - 1:
                at_tiles.pop(blk)

        # back units with readiness (in completed-front count)
        units = []
        for blk in range(NB):
            base = blk * TB * H
            for h in range(H):
                ready = base + (TB - 1) * H + h + 6
                for q in range(4):
                    units.append((ready, ("av", blk, h, q)))
            for ti in range(TB):
                units.append((base + TB * H + 6 + ti, ("op", blk, ti)))
        units.sort(key=lambda u: u[0])
        ucur = 0

        def emit_units(done, cap):
            nonlocal ucur
            n = 0
            while ucur < len(units) and units[ucur][0] <= done and n < cap:
                _, u = units[ucur]
                if u[0] == "av":
                    unit_av(u[1], u[2], u[3])
                else:
                    unit_oproj(u[1], u[2])
                ucur += 1
                n += 1

        for g in range(NT * H):
            front(g)
            emit_units(g + 1, 2)
        emit_units(10**9, 10**9)


_CACHE = {}


def _make_in_maps(inputs):
    import ml_dtypes

    bf16 = ml_dtypes.bfloat16
    nd = np.asarray(inputs["ndata"], np.float32)  # (B, N, F)
    ab = np.asarray(inputs["attn_bias"], np.float32)  # (B, N, N, H)
    am = np.asarray(inputs["attn_mask"])  # (B, N, N) bool
    B = nd.shape[0]
    ws = {}
    for w in ("q", "k", "v", "o"):
        ws[f"w{w}T"] = np.ascontiguousarray(
            np.asarray(inputs[f"W{w}"], np.float32).T
        ).astype(bf16)
        ws[f"b{w}"] = np.asarray(inputs[f"b{w}"], np.float32)
    in_maps = []
    for b in range(B):
        m = np.array(am[b])
        m[:, 0] = False
        biasT = np.where(
            m[:, None, :], np.float32(NEG), ab[b].transpose(0, 2, 1)
        ).astype(bf16)
        ndT = np.ascontiguousarray(nd[b].T).astype(bf16)
        entry = {"ndT": ndT, "biasT": biasT}
        entry.update(ws)
        in_maps.append(entry)
    return in_maps


def _get_nc():
    if "nc" not in _CACHE:
        _CACHE["nc"] = build_program()
    return _CACHE["nc"]


def _ensure_ntff_hook():
    """Shim antenv.axon_hooks (absent in this image) so trace=True works."""
    import types

    try:
        from antenv.axon_hooks import get_axon_ntff_profile_hook  # noqa: F401

        return
    except ImportError:
        pass
    import antenv

    mod = types.ModuleType("antenv.axon_hooks")
    _h = [None]
    mod.set_axon_ntff_profile_hook = lambda h: _h.__setitem__(0, h)
    mod.get_axon_ntff_profile_hook = lambda: _h[0]
    sys.modules["antenv.axon_hooks"] = mod
    antenv.axon_hooks = mod
    from trn_agent_boot.trn_boot import _ntff_profile_via_ctypes

    mod.set_axon_ntff_profile_hook(
        _ntff_profile_via_ctypes("/opt/axon/libaxon_pjrt.so")
    )


def run(inputs, trace=False):
    """Run on hardware; returns (output (B,N,F) f32, exec_time_ns or None)."""
    from concourse import bass_utils

    if trace:
        _ensure_ntff_hook()
    nc = _get_nc()
    in_maps = _make_in_maps(inputs)
    res = bass_utils.run_bass_kernel_spmd(
        nc, in_maps, core_ids=list(range(len(in_maps))), trace=trace
    )
    out = np.stack([r["out"] for r in res.results]).astype(np.float32)
    return out, res.exec_time_ns


def kernel(**inputs):
    out, _ = run(inputs, trace=False)
    return out
